# revision 24
# baseline (speedup 1.0000x reference)
"""Bahdanau-attention kernel for Trainium2 (8 NeuronCores, data-parallel).

Computation (per batch b):
  att  = relu(enc_out @ W_enc + b_enc + dec_out @ W_dec + b_dec)   [N, A]
  s    = att @ W_att (+ b_att, dropped: softmax-invariant)         [N]
  alpha = softmax(s)                                               [N]
  wenc = alpha @ enc_out                                           [E]
Returns (wenc [B, E], alpha [B, N]).

Layout strategy per core (8 batches/core):
  - enc_out shipped host-transposed as encT [BL, E, N] so the big matmul
    streams with E (contraction) on partitions; W_enc [E, A] is naturally
    partition-major. Output att is produced TRANSPOSED [A, N], which makes
    (b_enc + b_dec + dec@W_dec) a per-partition bias folded into a single
    ScalarE Relu-activation that also evicts PSUM.
  - scores via PE with W_att as a [128, 1] stationary.
  - softmax on single-partition rows (cheap), alpha broadcast to 128
    partitions via a PE outer-product with a ones vector.
  - weighted sum on VectorE: tensor_tensor_reduce(enc_chunk * alpha_bcast)
    per 128-row e-tile, reading the already-resident encT chunks.
  - Matmuls run as float32r (fp32 bits, ~1 cycle/row at free>=256,
    ~1.5e-4 rel err vs 2.4e-3 for bf16).
"""

import os

import numpy as np

import concourse.bacc as bacc
import concourse.mybir as mybir
import concourse.tile as tile
from concourse.bass_utils import run_bass_kernel_spmd

B, N, E, A, D = 64, 1024, 2048, 1024, 1024
NCORES = 8
BL = B // NCORES          # batches per core
NCHUNK = 512              # n-columns per streamed chunk
NCH = N // NCHUNK         # chunks per batch
EK = E // 128             # contraction tiles of the big matmul
AK = A // 128             # a-tiles (att rows / 128)
DK = D // 128             # contraction tiles of the dec matmul

F32 = mybir.dt.float32
F32R = mybir.dt.float32r
RELU = mybir.ActivationFunctionType.Relu
EXP = mybir.ActivationFunctionType.Exp
AX_X = mybir.AxisListType.X
OP_ADD = mybir.AluOpType.add
OP_MULT = mybir.AluOpType.mult
OP_MAX = mybir.AluOpType.max

_NC_CACHE = {}
LAST_RESULT = None


def _build():
    nc = bacc.Bacc()

    encT = nc.dram_tensor("encT", [BL, E, N], F32R, kind="ExternalInput")
    decT = nc.dram_tensor("decT", [D, BL], F32, kind="ExternalInput")
    W_enc = nc.dram_tensor("W_enc", [E, A], F32R, kind="ExternalInput")
    W_dec = nc.dram_tensor("W_dec", [D, A], F32, kind="ExternalInput")
    b_enc = nc.dram_tensor("b_enc", [A], F32, kind="ExternalInput")
    b_dec = nc.dram_tensor("b_dec", [A], F32, kind="ExternalInput")
    W_att = nc.dram_tensor("W_att", [A], F32R, kind="ExternalInput")
    w_out = nc.dram_tensor("w_out", [BL, E], F32, kind="ExternalOutput")
    alpha_out = nc.dram_tensor("alpha_out", [BL, N], F32, kind="ExternalOutput")

    encT_r = encT.rearrange("b (k p) n -> b p k n", p=128)
    W_enc_r = W_enc.rearrange("(k p) a -> p k a", p=128)
    W_dec_r = W_dec.rearrange("(k p) a -> p k a", p=128)
    decT_r = decT.rearrange("(k p) b -> p k b", p=128)
    b_enc_r = b_enc.rearrange("(k p) -> p k", p=128)
    b_dec_r = b_dec.rearrange("(k p) -> p k", p=128)
    W_att_r = W_att.rearrange("(k p) -> p k", p=128)
    w_out_r = w_out.rearrange("b (t p) -> b p t", p=128)

    with tile.TileContext(nc) as tc:
        with tc.tile_pool(name="resident", bufs=1) as res_pool:
            # weights + output stores go on the Scalar-engine HWDGE ring so
            # the Sync-engine ring carries only the enc stream: a late
            # batch-output DMA must never queue ahead of the next batch's
            # input loads.
            w_att_sb = res_pool.tile([128, AK], F32R, name="w_att_sb")
            nc.scalar.dma_start(w_att_sb, W_att_r)
            bias_sb = res_pool.tile([128, AK, BL], F32, name="bias_sb")
            w_enc_sb = res_pool.tile([128, EK, A], F32R, name="w_enc_sb")
            ones_sb = res_pool.tile([1, 128], F32, name="ones_sb")
            nc.gpsimd.memset(ones_sb, 1.0)

            # --- preamble: bias_sb[a, b] = (dec @ W_dec)[b, a] + b_enc + b_dec
            with (
                tc.tile_pool(name="pre", bufs=1) as pre_pool,
                tc.tile_pool(name="pre_ps", bufs=1, space="PSUM") as pre_ps,
            ):
                dec_sb = pre_pool.tile([128, DK, BL], F32, name="dec_sb")
                nc.gpsimd.dma_start(dec_sb, decT_r)
                benc_sb = pre_pool.tile([128, AK], F32, name="benc_sb")
                nc.gpsimd.dma_start(benc_sb, b_enc_r)
                bdec_sb = pre_pool.tile([128, AK], F32, name="bdec_sb")
                nc.gpsimd.dma_start(bdec_sb, b_dec_r)
                w_dec_sb = pre_pool.tile([128, DK, A], F32, name="w_dec_sb")
                nc.gpsimd.dma_start(w_dec_sb, W_dec_r)
                for e_k in range(EK):
                    nc.scalar.dma_start(w_enc_sb[:, e_k, :], W_enc_r[:, e_k, :])
                bsum_sb = pre_pool.tile([128, AK], F32, name="bsum_sb")
                nc.vector.tensor_add(bsum_sb, benc_sb, bdec_sb)
                for a_t in range(AK):
                    ps_d = pre_ps.tile([128, BL], F32, name="ps_d")
                    for d_k in range(DK):
                        nc.tensor.matmul(
                            ps_d,
                            w_dec_sb[:, d_k, a_t * 128:(a_t + 1) * 128],
                            dec_sb[:, d_k, :],
                            start=(d_k == 0), stop=(d_k == DK - 1),
                        )
                    nc.vector.tensor_scalar_add(
                        bias_sb[:, a_t, :], ps_d, bsum_sb[:, a_t:a_t + 1]
                    )

            with (
                tc.tile_pool(name="encp", bufs=3) as enc_pool,
                tc.tile_pool(name="attp", bufs=2) as att_pool,
                tc.tile_pool(name="rowp", bufs=2) as row_pool,
                tc.tile_pool(name="smallp", bufs=2) as small_pool,
                tc.tile_pool(name="ps_a", bufs=4, space="PSUM") as ps_a,
                tc.tile_pool(name="ps_s", bufs=2, space="PSUM") as ps_s,
                tc.tile_pool(name="ps_bc", bufs=2, space="PSUM") as ps_bc,
            ):
                # ---- online-softmax chunk pipeline ----
                # Chunk ci = (b, h).  Pass A produces scores for chunk ci in
                # PSUM; the chunk tail (emitted ~2 a-groups into the NEXT
                # chunk's pass A so nothing waits) computes the chunk max,
                # exp (chunk-referenced), and the p-weighted column sums via
                # VectorE, which releases the enc slab early.  Per batch the
                # two chunk sums are combined exactly:
                #   m01 = max(m0, m1); corr0 = exp(m0 - m01)
                #   Z = z0*corr0 + z1
                #   alpha = [p0*corr0, p1] / Z
                #   wenc  = (cs0*corr0 + cs1) / Z
                st = [dict() for _ in range(BL)]

                def emit_chunk_tail(ci):
                    b, h = divmod(ci, NCH)
                    s = st[b]
                    ps_sc = s["ps_sc"][h]
                    if h == 0:
                        m0 = small_pool.tile([1, 1], F32, name="m0")
                        nc.vector.tensor_reduce(m0, ps_sc, axis=AX_X, op=OP_MAX)
                        negm0 = small_pool.tile([1, 1], F32, name="negm0")
                        nc.vector.tensor_scalar_mul(negm0, m0, -1.0)
                        p0 = row_pool.tile([1, NCHUNK], F32, name="p0")
                        z0 = small_pool.tile([1, 1], F32, name="z0")
                        nc.scalar.activation(
                            p0, ps_sc, EXP, bias=negm0, scale=1.0, accum_out=z0
                        )
                        s.update(m0=m0, p0=p0, z0=z0)
                        ph = p0
                    else:
                        m1 = small_pool.tile([1, 1], F32, name="m1")
                        nc.vector.tensor_reduce(m1, ps_sc, axis=AX_X, op=OP_MAX)
                        m01 = small_pool.tile([1, 1], F32, name="m01")
                        nc.vector.tensor_max(m01, m1, s["m0"])
                        negm01 = small_pool.tile([1, 1], F32, name="negm01")
                        nc.vector.tensor_scalar_mul(negm01, m01, -1.0)
                        p1 = row_pool.tile([1, NCHUNK], F32, name="p1")
                        z1 = small_pool.tile([1, 1], F32, name="z1")
                        nc.scalar.activation(
                            p1, ps_sc, EXP, bias=negm01, scale=1.0, accum_out=z1
                        )
                        # ci2 = [corr0, invz] on partition 0
                        ci2 = small_pool.tile([1, 2], F32, name="ci2")
                        nc.scalar.activation(
                            ci2[:, 0:1], s["m0"], EXP, bias=negm01, scale=1.0
                        )
                        zc = small_pool.tile([1, 1], F32, name="zc")
                        nc.vector.scalar_tensor_tensor(
                            zc, s["z0"], ci2[:, 0:1], z1, OP_MULT, OP_ADD
                        )
                        nc.vector.reciprocal(ci2[:, 1:2], zc)
                        c0 = small_pool.tile([1, 1], F32, name="c0")
                        nc.vector.tensor_tensor(
                            c0, ci2[:, 1:2], ci2[:, 0:1], op=OP_MULT
                        )
                        alpha_sb = row_pool.tile([1, N], F32, name="alpha_sb")
                        nc.vector.tensor_scalar_mul(
                            alpha_sb[:, 0:NCHUNK], s["p0"], c0
                        )
                        nc.vector.tensor_scalar_mul(
                            alpha_sb[:, NCHUNK:N], p1, ci2[:, 1:2]
                        )
                        nc.scalar.dma_start(alpha_out[b:b + 1, :], alpha_sb)
                        # broadcast [corr0, invz] to all partitions (tiny,
                        # off the critical path - only w_out depends on it)
                        bc_ci = small_pool.tile([128, 2], F32, name="bc_ci")
                        nc.gpsimd.partition_broadcast(bc_ci, ci2)
                        s.update(p1=p1, bc_ci=bc_ci)
                        ph = p1

                    # p-weighted column sums for this chunk on VectorE
                    ps_al = ps_bc.tile([128, NCHUNK], F32, name="ps_al")
                    nc.tensor.matmul(ps_al, ones_sb, ph, start=True, stop=True)
                    cs = small_pool.tile([128, EK], F32, name=f"cs{h}")
                    scratch = att_pool.tile([128, NCHUNK], F32, name="scratch")
                    for e_t in range(EK):
                        nc.vector.scalar_tensor_tensor(
                            out=scratch,
                            in0=s["enc"][h][:, e_t, :].bitcast(F32),
                            scalar=1.0,
                            in1=ps_al,
                            op0=OP_MULT,
                            op1=OP_MULT,
                            accum_out=cs[:, e_t:e_t + 1],
                        )
                    s[f"cs{h}"] = cs

                    if h == 1:
                        bc_ci = s["bc_ci"]
                        w_tmp = small_pool.tile([128, EK], F32, name="w_tmp")
                        nc.vector.scalar_tensor_tensor(
                            w_tmp, s["cs0"], bc_ci[:, 0:1], cs, OP_MULT, OP_ADD
                        )
                        w_acc = small_pool.tile([128, EK], F32, name="w_acc")
                        nc.vector.tensor_scalar_mul(w_acc, w_tmp, bc_ci[:, 1:2])
                        nc.scalar.dma_start(w_out_r[b], w_acc)

                for b in range(BL):
                    s = st[b]
                    s["enc"] = []
                    for h in range(NCH):
                        ec = enc_pool.tile([128, EK, NCHUNK], F32R, name="enc_ch")
                        nc.sync.dma_start(
                            ec, encT_r[b, :, :, h * NCHUNK:(h + 1) * NCHUNK]
                        )
                        s["enc"].append(ec)
                    s["ps_sc"] = []
                    for h in range(NCH):
                        ci = b * NCH + h
                        ps_sc = ps_s.tile([1, NCHUNK], F32, name="ps_sc")
                        s["ps_sc"].append(ps_sc)
                        for a_t in range(AK):
                            ps_acc = ps_a.tile([128, NCHUNK], F32, name="ps_acc")
                            for e_k in range(EK):
                                nc.tensor.matmul(
                                    ps_acc,
                                    w_enc_sb[:, e_k, a_t * 128:(a_t + 1) * 128],
                                    s["enc"][h][:, e_k, :],
                                    start=(e_k == 0), stop=(e_k == EK - 1),
                                )
                            att_t = att_pool.tile([128, NCHUNK], F32R, name="att_t")
                            nc.scalar.activation(
                                att_t, ps_acc, RELU, bias=bias_sb[:, a_t, b:b + 1]
                            )
                            nc.tensor.matmul(
                                ps_sc,
                                w_att_sb[:, a_t:a_t + 1],
                                att_t,
                                start=(a_t == 0), stop=(a_t == AK - 1),
                            )
                            if ci > 0 and a_t == 2:
                                emit_chunk_tail(ci - 1)
                emit_chunk_tail(BL * NCH - 1)

    nc.compile()
    return nc


def _get_nc():
    if "nc" not in _NC_CACHE:
        _NC_CACHE["nc"] = _build()
    return _NC_CACHE["nc"]


def kernel(enc_out, dec_out, W_enc, b_enc, W_dec, b_dec, W_att, b_att=None,
           **_unused):
    global LAST_RESULT
    enc_out = np.ascontiguousarray(np.asarray(enc_out, dtype=np.float32))
    dec_out = np.ascontiguousarray(np.asarray(dec_out, dtype=np.float32))
    W_enc = np.ascontiguousarray(np.asarray(W_enc, dtype=np.float32))
    W_dec = np.ascontiguousarray(np.asarray(W_dec, dtype=np.float32))
    b_enc = np.ascontiguousarray(np.asarray(b_enc, dtype=np.float32))
    b_dec = np.ascontiguousarray(np.asarray(b_dec, dtype=np.float32))
    W_att = np.ascontiguousarray(np.asarray(W_att, dtype=np.float32))

    nc = _get_nc()
    in_maps = []
    for c in range(NCORES):
        sl = slice(c * BL, (c + 1) * BL)
        in_maps.append({
            "encT": np.ascontiguousarray(enc_out[sl].transpose(0, 2, 1)),
            "decT": np.ascontiguousarray(dec_out[sl].T),
            "W_enc": W_enc,
            "W_dec": W_dec,
            "b_enc": b_enc,
            "b_dec": b_dec,
            "W_att": W_att,
        })

    trace = bool(os.environ.get("KERNEL_TRACE"))
    res = run_bass_kernel_spmd(
        nc, in_maps, core_ids=list(range(NCORES)), trace=trace
    )
    LAST_RESULT = res

    weighted = np.concatenate([r["w_out"] for r in res.results], axis=0)
    alpha = np.concatenate([r["alpha_out"] for r in res.results], axis=0)
    return weighted.astype(np.float32), alpha.astype(np.float32)


# revision 26
# speedup vs baseline: 1.2337x; 1.2337x over previous
"""Bahdanau-attention kernel for Trainium2 (8 NeuronCores, data-parallel).

Computation (per batch b):
  att  = relu(enc_out @ W_enc + b_enc + dec_out @ W_dec + b_dec)   [N, A]
  s    = att @ W_att (+ b_att, dropped: softmax-invariant)         [N]
  alpha = softmax(s)                                               [N]
  wenc = alpha @ enc_out                                           [E]
Returns (wenc [B, E], alpha [B, N]).

Layout strategy per core (8 batches/core):
  - enc_out shipped host-transposed as encT [BL, E, N] so the big matmul
    streams with E (contraction) on partitions; W_enc [E, A] is naturally
    partition-major. Output att is produced TRANSPOSED [A, N], which makes
    (b_enc + b_dec + dec@W_dec) a per-partition bias folded into a single
    ScalarE Relu-activation that also evicts PSUM.
  - scores via PE with W_att as a [128, 1] stationary.
  - softmax on single-partition rows (cheap), alpha broadcast to 128
    partitions via a PE outer-product with a ones vector.
  - weighted sum on VectorE: tensor_tensor_reduce(enc_chunk * alpha_bcast)
    per 128-row e-tile, reading the already-resident encT chunks.
  - Matmuls run as float32r (fp32 bits, ~1 cycle/row at free>=256,
    ~1.5e-4 rel err vs 2.4e-3 for bf16).
"""

import os

import numpy as np

import concourse.bacc as bacc
import concourse.mybir as mybir
import concourse.tile as tile
from concourse.bass_utils import run_bass_kernel_spmd

B, N, E, A, D = 64, 1024, 2048, 1024, 1024
NCORES = 8
BL = B // NCORES          # batches per core
NCHUNK = 512              # n-columns per streamed chunk
NCH = N // NCHUNK         # chunks per batch
EK = E // 128             # contraction tiles of the big matmul
AK = A // 128             # a-tiles (att rows / 128)
DK = D // 128             # contraction tiles of the dec matmul

F32 = mybir.dt.float32
F32R = mybir.dt.float32r
RELU = mybir.ActivationFunctionType.Relu
EXP = mybir.ActivationFunctionType.Exp
AX_X = mybir.AxisListType.X
OP_ADD = mybir.AluOpType.add
OP_MULT = mybir.AluOpType.mult
OP_MAX = mybir.AluOpType.max

_NC_CACHE = {}
LAST_RESULT = None


def _build():
    nc = bacc.Bacc()

    encT = nc.dram_tensor("encT", [BL, E, N], F32R, kind="ExternalInput")
    decT = nc.dram_tensor("decT", [D, BL], F32, kind="ExternalInput")
    W_enc = nc.dram_tensor("W_enc", [E, A], F32R, kind="ExternalInput")
    W_dec = nc.dram_tensor("W_dec", [D, A], F32, kind="ExternalInput")
    b_enc = nc.dram_tensor("b_enc", [A], F32, kind="ExternalInput")
    b_dec = nc.dram_tensor("b_dec", [A], F32, kind="ExternalInput")
    W_att = nc.dram_tensor("W_att", [A], F32R, kind="ExternalInput")
    w_out = nc.dram_tensor("w_out", [BL, E], F32, kind="ExternalOutput")
    alpha_out = nc.dram_tensor("alpha_out", [BL, N], F32, kind="ExternalOutput")

    encT_r = encT.rearrange("b (k p) n -> b p k n", p=128)
    W_enc_r = W_enc.rearrange("(k p) a -> p k a", p=128)
    W_dec_r = W_dec.rearrange("(k p) a -> p k a", p=128)
    decT_r = decT.rearrange("(k p) b -> p k b", p=128)
    b_enc_r = b_enc.rearrange("(k p) -> p k", p=128)
    b_dec_r = b_dec.rearrange("(k p) -> p k", p=128)
    W_att_r = W_att.rearrange("(k p) -> p k", p=128)
    w_out_r = w_out.rearrange("b (t p) -> b p t", p=128)

    with tile.TileContext(nc) as tc:
        with tc.tile_pool(name="resident", bufs=1) as res_pool:
            # weights + output stores go on the Scalar-engine HWDGE ring so
            # the Sync-engine ring carries only the enc stream: a late
            # batch-output DMA must never queue ahead of the next batch's
            # input loads.
            w_att_sb = res_pool.tile([128, AK], F32R, name="w_att_sb")
            nc.scalar.dma_start(w_att_sb, W_att_r)
            bias_sb = res_pool.tile([128, AK, BL], F32, name="bias_sb")
            w_enc_sb = res_pool.tile([128, EK, A], F32R, name="w_enc_sb")
            ones_sb = res_pool.tile([1, 128], F32, name="ones_sb")
            nc.gpsimd.memset(ones_sb, 1.0)

            # --- preamble: bias_sb[a, b] = (dec @ W_dec)[b, a] + b_enc + b_dec
            with (
                tc.tile_pool(name="pre", bufs=1) as pre_pool,
                tc.tile_pool(name="pre_ps", bufs=1, space="PSUM") as pre_ps,
            ):
                dec_sb = pre_pool.tile([128, DK, BL], F32, name="dec_sb")
                nc.gpsimd.dma_start(dec_sb, decT_r)
                benc_sb = pre_pool.tile([128, AK], F32, name="benc_sb")
                nc.gpsimd.dma_start(benc_sb, b_enc_r)
                bdec_sb = pre_pool.tile([128, AK], F32, name="bdec_sb")
                nc.gpsimd.dma_start(bdec_sb, b_dec_r)
                # W_dec and W_enc tiles interleaved on the scalar ring: the
                # dec matmuls (d_k-outer, one PSUM bank per a-tile - all 8
                # banks are free during the preamble) chase the W_dec
                # arrivals while the big matmuls chase W_enc.
                w_dec_sb = pre_pool.tile([128, DK, A], F32, name="w_dec_sb")
                for d_k in range(DK):
                    nc.scalar.dma_start(w_dec_sb[:, d_k, :], W_dec_r[:, d_k, :])
                    nc.scalar.dma_start(
                        w_enc_sb[:, d_k, :], W_enc_r[:, d_k, :]
                    )
                for e_k in range(DK, EK):
                    nc.scalar.dma_start(w_enc_sb[:, e_k, :], W_enc_r[:, e_k, :])
                bsum_sb = pre_pool.tile([128, AK], F32, name="bsum_sb")
                nc.vector.tensor_add(bsum_sb, benc_sb, bdec_sb)
                ps_d = [
                    pre_ps.tile([128, BL], F32, name=f"ps_d{a_t}")
                    for a_t in range(AK)
                ]
                for d_k in range(DK):
                    for a_t in range(AK):
                        nc.tensor.matmul(
                            ps_d[a_t],
                            w_dec_sb[:, d_k, a_t * 128:(a_t + 1) * 128],
                            dec_sb[:, d_k, :],
                            start=(d_k == 0), stop=(d_k == DK - 1),
                        )
                for a_t in range(AK):
                    nc.vector.tensor_scalar_add(
                        bias_sb[:, a_t, :], ps_d[a_t], bsum_sb[:, a_t:a_t + 1]
                    )

            with (
                tc.tile_pool(name="encp", bufs=3) as enc_pool,
                tc.tile_pool(name="attp", bufs=2) as att_pool,
                tc.tile_pool(name="rowp", bufs=2) as row_pool,
                tc.tile_pool(name="smallp", bufs=2) as small_pool,
                tc.tile_pool(name="ps_a", bufs=4, space="PSUM") as ps_a,
                tc.tile_pool(name="ps_s", bufs=2, space="PSUM") as ps_s,
                tc.tile_pool(name="ps_bc", bufs=2, space="PSUM") as ps_bc,
            ):
                # ---- online-softmax chunk pipeline ----
                # Chunk ci = (b, h).  Pass A produces scores for chunk ci in
                # PSUM; the chunk tail (emitted ~2 a-groups into the NEXT
                # chunk's pass A so nothing waits) computes the chunk max,
                # exp (chunk-referenced), and the p-weighted column sums via
                # VectorE, which releases the enc slab early.  Per batch the
                # two chunk sums are combined exactly:
                #   m01 = max(m0, m1); corr0 = exp(m0 - m01)
                #   Z = z0*corr0 + z1
                #   alpha = [p0*corr0, p1] / Z
                #   wenc  = (cs0*corr0 + cs1) / Z
                st = [dict() for _ in range(BL)]

                def emit_chunk_tail(ci):
                    b, h = divmod(ci, NCH)
                    s = st[b]
                    ps_sc = s["ps_sc"][h]
                    if h == 0:
                        m0 = small_pool.tile([1, 1], F32, name="m0")
                        nc.vector.tensor_reduce(m0, ps_sc, axis=AX_X, op=OP_MAX)
                        negm0 = small_pool.tile([1, 1], F32, name="negm0")
                        nc.vector.tensor_scalar_mul(negm0, m0, -1.0)
                        p0 = row_pool.tile([1, NCHUNK], F32, name="p0")
                        z0 = small_pool.tile([1, 1], F32, name="z0")
                        nc.scalar.activation(
                            p0, ps_sc, EXP, bias=negm0, scale=1.0, accum_out=z0
                        )
                        s.update(m0=m0, p0=p0, z0=z0)
                        ph = p0
                    else:
                        m1 = small_pool.tile([1, 1], F32, name="m1")
                        nc.vector.tensor_reduce(m1, ps_sc, axis=AX_X, op=OP_MAX)
                        m01 = small_pool.tile([1, 1], F32, name="m01")
                        nc.vector.tensor_max(m01, m1, s["m0"])
                        negm01 = small_pool.tile([1, 1], F32, name="negm01")
                        nc.vector.tensor_scalar_mul(negm01, m01, -1.0)
                        p1 = row_pool.tile([1, NCHUNK], F32, name="p1")
                        z1 = small_pool.tile([1, 1], F32, name="z1")
                        nc.scalar.activation(
                            p1, ps_sc, EXP, bias=negm01, scale=1.0, accum_out=z1
                        )
                        # ci2 = [corr0, invz] on partition 0
                        ci2 = small_pool.tile([1, 2], F32, name="ci2")
                        nc.scalar.activation(
                            ci2[:, 0:1], s["m0"], EXP, bias=negm01, scale=1.0
                        )
                        zc = small_pool.tile([1, 1], F32, name="zc")
                        nc.vector.scalar_tensor_tensor(
                            zc, s["z0"], ci2[:, 0:1], z1, OP_MULT, OP_ADD
                        )
                        nc.vector.reciprocal(ci2[:, 1:2], zc)
                        c0 = small_pool.tile([1, 1], F32, name="c0")
                        nc.vector.tensor_tensor(
                            c0, ci2[:, 1:2], ci2[:, 0:1], op=OP_MULT
                        )
                        alpha_sb = row_pool.tile([1, N], F32, name="alpha_sb")
                        nc.vector.tensor_scalar_mul(
                            alpha_sb[:, 0:NCHUNK], s["p0"], c0
                        )
                        nc.vector.tensor_scalar_mul(
                            alpha_sb[:, NCHUNK:N], p1, ci2[:, 1:2]
                        )
                        nc.scalar.dma_start(alpha_out[b:b + 1, :], alpha_sb)
                        # broadcast [corr0, invz] to all partitions (tiny,
                        # off the critical path - only w_out depends on it)
                        bc_ci = small_pool.tile([128, 2], F32, name="bc_ci")
                        nc.gpsimd.partition_broadcast(bc_ci, ci2)
                        s.update(p1=p1, bc_ci=bc_ci)
                        ph = p1

                    # p-weighted column sums for this chunk on VectorE
                    ps_al = ps_bc.tile([128, NCHUNK], F32, name="ps_al")
                    nc.tensor.matmul(ps_al, ones_sb, ph, start=True, stop=True)
                    cs = small_pool.tile([128, EK], F32, name=f"cs{h}")
                    scratch = att_pool.tile([128, NCHUNK], F32, name="scratch")
                    for e_t in range(EK):
                        nc.vector.scalar_tensor_tensor(
                            out=scratch,
                            in0=s["enc"][h][:, e_t, :].bitcast(F32),
                            scalar=1.0,
                            in1=ps_al,
                            op0=OP_MULT,
                            op1=OP_MULT,
                            accum_out=cs[:, e_t:e_t + 1],
                        )
                    s[f"cs{h}"] = cs

                    if h == 1:
                        bc_ci = s["bc_ci"]
                        w_tmp = small_pool.tile([128, EK], F32, name="w_tmp")
                        nc.vector.scalar_tensor_tensor(
                            w_tmp, s["cs0"], bc_ci[:, 0:1], cs, OP_MULT, OP_ADD
                        )
                        w_acc = small_pool.tile([128, EK], F32, name="w_acc")
                        nc.vector.tensor_scalar_mul(w_acc, w_tmp, bc_ci[:, 1:2])
                        nc.scalar.dma_start(w_out_r[b], w_acc)

                for b in range(BL):
                    s = st[b]
                    s["enc"] = []
                    for h in range(NCH):
                        ec = enc_pool.tile([128, EK, NCHUNK], F32R, name="enc_ch")
                        nc.sync.dma_start(
                            ec, encT_r[b, :, :, h * NCHUNK:(h + 1) * NCHUNK]
                        )
                        s["enc"].append(ec)
                    s["ps_sc"] = []
                    for h in range(NCH):
                        ci = b * NCH + h
                        ps_sc = ps_s.tile([1, NCHUNK], F32, name="ps_sc")
                        s["ps_sc"].append(ps_sc)
                        for a_t in range(AK):
                            ps_acc = ps_a.tile([128, NCHUNK], F32, name="ps_acc")
                            for e_k in range(EK):
                                nc.tensor.matmul(
                                    ps_acc,
                                    w_enc_sb[:, e_k, a_t * 128:(a_t + 1) * 128],
                                    s["enc"][h][:, e_k, :],
                                    start=(e_k == 0), stop=(e_k == EK - 1),
                                )
                            att_t = att_pool.tile([128, NCHUNK], F32R, name="att_t")
                            nc.scalar.activation(
                                att_t, ps_acc, RELU, bias=bias_sb[:, a_t, b:b + 1]
                            )
                            nc.tensor.matmul(
                                ps_sc,
                                w_att_sb[:, a_t:a_t + 1],
                                att_t,
                                start=(a_t == 0), stop=(a_t == AK - 1),
                            )
                            if ci > 0 and a_t == 2:
                                emit_chunk_tail(ci - 1)
                emit_chunk_tail(BL * NCH - 1)

    nc.compile()
    return nc


def _get_nc():
    if "nc" not in _NC_CACHE:
        _NC_CACHE["nc"] = _build()
    return _NC_CACHE["nc"]


def kernel(enc_out, dec_out, W_enc, b_enc, W_dec, b_dec, W_att, b_att=None,
           **_unused):
    global LAST_RESULT
    enc_out = np.ascontiguousarray(np.asarray(enc_out, dtype=np.float32))
    dec_out = np.ascontiguousarray(np.asarray(dec_out, dtype=np.float32))
    W_enc = np.ascontiguousarray(np.asarray(W_enc, dtype=np.float32))
    W_dec = np.ascontiguousarray(np.asarray(W_dec, dtype=np.float32))
    b_enc = np.ascontiguousarray(np.asarray(b_enc, dtype=np.float32))
    b_dec = np.ascontiguousarray(np.asarray(b_dec, dtype=np.float32))
    W_att = np.ascontiguousarray(np.asarray(W_att, dtype=np.float32))

    nc = _get_nc()
    in_maps = []
    for c in range(NCORES):
        sl = slice(c * BL, (c + 1) * BL)
        in_maps.append({
            "encT": np.ascontiguousarray(enc_out[sl].transpose(0, 2, 1)),
            "decT": np.ascontiguousarray(dec_out[sl].T),
            "W_enc": W_enc,
            "W_dec": W_dec,
            "b_enc": b_enc,
            "b_dec": b_dec,
            "W_att": W_att,
        })

    trace = bool(os.environ.get("KERNEL_TRACE"))
    res = run_bass_kernel_spmd(
        nc, in_maps, core_ids=list(range(NCORES)), trace=trace
    )
    LAST_RESULT = res

    weighted = np.concatenate([r["w_out"] for r in res.results], axis=0)
    alpha = np.concatenate([r["alpha_out"] for r in res.results], axis=0)
    return weighted.astype(np.float32), alpha.astype(np.float32)
